# revision 4
# baseline (speedup 1.0000x reference)
"""BitLinear158 Trainium2 kernel (per-core body + host driver).

Per core: x_shard [M_LOC, K] bf16 -> per-token int8 quant -> bf16 matmul
against host-unpacked ternary wT [K, N] -> rescale -> y [M_LOC, N] bf16.

v3 pipeline (quant AFTER transpose; no DRAM round-trip for activations):
  amax path (natural layout, per m-tile):
    x tile [128,2048]  <- scalar-ring DMA
    amax = DVE reduce(abs max); s_bf = bf16(127/amax); r = 1/s/ws (f32)
    s stored to s_dram row; per chunk: s_row load + gpsimd partition
    broadcast -> s_bc [128, cw] f32
  transpose path (independent of quant; src is the raw x input in DRAM):
    xT[kc] [128, cw] bf16 <- sync-ring xbar DMA transpose of x slice
  quant (k-layout, per (chunk, kc)):
    xq8  = DVE tensor_tensor(mult, xT, s_bc) -> int8 (RNE+saturate)
    xqbf = DVE tensor_copy(xq8) -> bf16
  weights: wT int8 [K, N] in DRAM; SWDGE (gpsimd) per-kc DMA-cast loads
    int8 -> bf16 into SBUF (first k-chunk ready ~1us after start)
  matmul:  PSUM [128m,512n] f32 += xqbf[kc][:,mi].T @ wT[:,kc,nt] (16 kc)
  rescale: y_sb = ACT activation(Copy, scale=r) -> bf16 ; y <- gpsimd DMA
"""

import sys

sys.path.insert(0, "/opt/trn_rl_repo")

from contextlib import ExitStack

import numpy as np
import ml_dtypes

import concourse.bass as bass
import concourse.tile as tile
from concourse import bacc, mybir
from concourse import bass_utils

P = 128
M_LOC = 4096      # tokens per core
K = 2048          # in features
N = 2048          # out features
KC = K // P       # 16 k-chunks
NT = M_LOC // P   # 32 m-tiles per core
# chunk sizes in m-tiles: small first chunks so the PE starts early
CHUNK_MTS = [1, 1, 2, 3, 4, 4, 4, 4, 4, 5]
assert sum(CHUNK_MTS) == NT
CHUNK_STARTS = [sum(CHUNK_MTS[:i]) for i in range(len(CHUNK_MTS))]
CHUNKS = len(CHUNK_MTS)
MAX_CHUNK_MT = max(CHUNK_MTS)
N_TILE = 512
NTN = N // N_TILE                  # 4
N_CORES = 8

BF16 = mybir.dt.bfloat16
F32 = mybir.dt.float32
I8 = mybir.dt.int8


def build_kernel():
    nc = bacc.Bacc("TRN2", target_bir_lowering=False, debug=False, num_devices=N_CORES)
    x = nc.dram_tensor("x", [M_LOC, K], BF16, kind="ExternalInput").ap()
    wTi8 = nc.dram_tensor("wTi8", [K, N], I8, kind="ExternalInput").ap()
    inv_ws = nc.dram_tensor("inv_ws", [1, 1], F32, kind="ExternalInput").ap()
    y = nc.dram_tensor("y", [M_LOC, N], BF16, kind="ExternalOutput").ap()

    x_tiled = x.rearrange("(t p) k -> t p k", p=P)
    y_tiled = y.rearrange("(t p) n -> t p n", p=P)
    wT_tiled = wTi8.rearrange("(c p) n -> p c n", p=P)

    with tile.TileContext(nc) as tc, ExitStack() as ctx:
        wbuf = ctx.enter_context(tc.tile_pool(name="wbuf", bufs=1))
        xin = ctx.enter_context(tc.tile_pool(name="xin", bufs=4))
        xT_pool = ctx.enter_context(tc.tile_pool(name="xT", bufs=2))
        xq8_pool = ctx.enter_context(tc.tile_pool(name="xq8", bufs=2))
        xqbf_pool = ctx.enter_context(tc.tile_pool(name="xqbf", bufs=2))
        stat = ctx.enter_context(tc.tile_pool(name="stat", bufs=4))
        rbuf = ctx.enter_context(tc.tile_pool(name="rbuf", bufs=1))
        srow_pool = ctx.enter_context(tc.tile_pool(name="srow", bufs=1))
        sbc_pool = ctx.enter_context(tc.tile_pool(name="sbc", bufs=2))
        yout = ctx.enter_context(tc.tile_pool(name="yout", bufs=4))
        psum = ctx.enter_context(tc.tile_pool(name="psum", bufs=8, space="PSUM"))
        dram = ctx.enter_context(tc.tile_pool(name="dram", bufs=1, space="DRAM"))

        # weights: per-kc SWDGE cast loads (int8 DRAM -> bf16 SBUF); the
        # first k-chunk lands ~1us in so matmuls are never weight-gated.
        wt = wbuf.tile([P, KC, N], BF16)
        for kc in range(KC):
            nc.gpsimd.dma_start(wt[:, kc, :], wT_tiled[:, kc, :])

        # 1/weight_scale, broadcast to all partitions once
        iws1 = stat.tile([1, 1], F32, tag="iws1", name="iws1")
        nc.scalar.dma_start(iws1[:], inv_ws)
        iws_bc = stat.tile([P, 1], F32, tag="iws_bc", name="iws_bc")
        nc.gpsimd.partition_broadcast(iws_bc[:], iws1[:])

        r_all = rbuf.tile([P, NT], F32)
        s_dram = dram.tile([1, NT * P], F32, tag="s_dram", name="s_dram")

        # x loads for the amax path run on the scalar (ACT) HWDGE ring
        xt_tiles = {}

        def load_tile(mt):
            if mt >= NT or mt in xt_tiles:
                return
            xt = xin.tile([P, K], BF16, tag="xt", name="xt")
            nc.scalar.dma_start(xt[:], x_tiled[mt])
            xt_tiles[mt] = xt

        def amax_tile(mt):
            load_tile(mt)
            load_tile(mt + 1)
            load_tile(mt + 2)
            xt = xt_tiles.pop(mt)

            amax = stat.tile([P, 1], F32, tag="amax", name="amax")
            nc.vector.tensor_reduce(
                amax[:], xt[:], axis=mybir.AxisListType.X,
                op=mybir.AluOpType.max, apply_absolute_value=True,
            )
            nc.vector.tensor_scalar_max(amax[:], amax[:], 1e-5)
            q = stat.tile([P, 1], F32, tag="q", name="q")
            nc.vector.reciprocal(q[:], amax[:])
            s_bf = stat.tile([P, 1], BF16, tag="s_bf", name="s_bf")
            nc.vector.tensor_scalar_mul(s_bf[:], q[:], 127.0)
            s_f32 = stat.tile([P, 1], F32, tag="s_f32", name="s_f32")
            nc.vector.tensor_copy(s_f32[:], s_bf[:])
            r0 = stat.tile([P, 1], F32, tag="r0", name="r0")
            nc.vector.reciprocal(r0[:], s_f32[:])
            nc.vector.tensor_tensor(
                r_all[:, mt : mt + 1], r0[:], iws_bc[:], op=mybir.AluOpType.mult
            )
            # store s as a row chunk of s_dram (DRAM-side AP transposed)
            nc.scalar.dma_start(
                s_dram[0:1, mt * P : (mt + 1) * P].rearrange("a b -> b a"),
                s_f32[:],
            )

        def srow_sbc(c):
            cw = CHUNK_MTS[c] * P
            c0 = CHUNK_STARTS[c] * P
            srow = srow_pool.tile([1, MAX_CHUNK_MT * P], F32, tag="srow", name="srow")
            nc.scalar.dma_start(srow[0:1, :cw], s_dram[0:1, c0 : c0 + cw])
            sbc = sbc_pool.tile([P, MAX_CHUNK_MT * P], F32, tag="sbc", name="sbc")
            nc.gpsimd.partition_broadcast(sbc[:, :cw], srow[0:1, :cw])
            return sbc

        def transpose_chunk(c):
            cw = CHUNK_MTS[c] * P
            m0 = CHUNK_STARTS[c] * P
            tiles = []
            for kc in range(KC):
                tt = xT_pool.tile(
                    [P, MAX_CHUNK_MT * P], BF16, tag=f"xT{kc}", name=f"xT{kc}"
                )
                nc.sync.dma_start_transpose(
                    tt[:, :cw],
                    x[m0 : m0 + cw, kc * P : (kc + 1) * P],
                )
                tiles.append(tt)
            return tiles

        def quant_chunk(c, xT, sbc):
            cw = CHUNK_MTS[c] * P
            tiles = []
            for kc in range(KC):
                q8 = xq8_pool.tile(
                    [P, MAX_CHUNK_MT * P], I8, tag=f"xq8{kc}", name=f"xq8{kc}"
                )
                nc.vector.tensor_tensor(
                    q8[:, :cw], xT[kc][:, :cw], sbc[:, :cw], op=mybir.AluOpType.mult
                )
                qb = xqbf_pool.tile(
                    [P, MAX_CHUNK_MT * P], BF16, tag=f"xqbf{kc}", name=f"xqbf{kc}"
                )
                nc.vector.tensor_copy(qb[:, :cw], q8[:, :cw])
                tiles.append(qb)
            return tiles

        def matmul_mtile(c, mi, xqbf):
            mt = CHUNK_STARTS[c] + mi
            y_sb = yout.tile([P, N], BF16, tag="y_sb", name="y_sb")
            for nt in range(NTN):
                ps = psum.tile([P, N_TILE], F32, tag="ps", name="ps")
                for kc in range(KC):
                    nc.tensor.matmul(
                        ps[:],
                        xqbf[kc][:, mi * P : (mi + 1) * P],
                        wt[:, kc, nt * N_TILE : (nt + 1) * N_TILE],
                        start=(kc == 0),
                        stop=(kc == KC - 1),
                    )
                nc.scalar.activation(
                    y_sb[:, nt * N_TILE : (nt + 1) * N_TILE],
                    ps[:],
                    mybir.ActivationFunctionType.Copy,
                    scale=r_all[:, mt : mt + 1],
                )
            nc.gpsimd.dma_start(y_tiled[mt], y_sb[:])

        def amax_chunk(c):
            for mt in range(CHUNK_STARTS[c], CHUNK_STARTS[c] + CHUNK_MTS[c]):
                amax_tile(mt)

        # software pipeline: transposes/quant of chunk c+1 and amax of
        # chunk c+2 run during chunk c's matmuls.
        amax_chunk(0)
        amax_chunk(1)
        sbc0 = srow_sbc(0)
        xq_map = {0: quant_chunk(0, transpose_chunk(0), sbc0)}
        for c in range(CHUNKS):
            if c + 1 < CHUNKS:
                sbc = srow_sbc(c + 1)
                xq_map[c + 1] = quant_chunk(c + 1, transpose_chunk(c + 1), sbc)
            if c + 2 < CHUNKS:
                amax_chunk(c + 2)
            for mi in range(CHUNK_MTS[c]):
                matmul_mtile(c, mi, xq_map[c])
            del xq_map[c]

    nc.compile()
    return nc


def unpack_wT_i8(packed_weight: np.ndarray) -> np.ndarray:
    planes = [((packed_weight >> (2 * i)) & 3) for i in range(4)]
    w = np.concatenate(planes, axis=0).astype(np.int8) - 1  # [N, K] ternary
    return np.ascontiguousarray(w.T)  # [K, N] int8


_CACHE = {}


def run(x: np.ndarray, packed_weight: np.ndarray, weight_scale: np.ndarray,
        trace: bool = False, tmpdir=None):
    """x: [B, S, K] bf16 -> y [B, S, N] bf16 (full, unsharded)."""
    if "nc" not in _CACHE:
        _CACHE["nc"] = build_kernel()
    nc = _CACHE["nc"]

    B, S, D = x.shape
    M = B * S
    assert M == M_LOC * N_CORES and D == K
    wT = unpack_wT_i8(packed_weight)
    iws = np.asarray(
        1.0 / np.float32(np.asarray(weight_scale).reshape(-1)[0]), dtype=np.float32
    ).reshape(1, 1)
    shards = np.ascontiguousarray(np.asarray(x).reshape(N_CORES, M_LOC, K))
    in_maps = [
        {"x": shards[i], "wTi8": wT, "inv_ws": iws} for i in range(N_CORES)
    ]
    res = bass_utils.run_bass_kernel_spmd(
        nc, in_maps, core_ids=list(range(N_CORES)), trace=trace, tmpdir=tmpdir
    )
    y = np.stack([res.results[i]["y"] for i in range(N_CORES)], axis=0)
    return y.reshape(B, S, N), res


def kernel(x, packed_weight, weight_scale):
    """Harness entrypoint: FULL inputs -> FULL output.

    x: [4, 8192, 2048] bf16; packed_weight: [512, 2048] uint8;
    weight_scale: [1] bf16.  Returns [4, 8192, 2048] bf16.
    Sharding: data-parallel over tokens across the 8 NeuronCores;
    the (host-unpacked) ternary weight is replicated.
    """
    x = np.asarray(x)
    packed_weight = np.asarray(packed_weight)
    weight_scale = np.asarray(weight_scale)
    y, _ = run(x, packed_weight, weight_scale)
    return y


# revision 8
# speedup vs baseline: 1.3825x; 1.3825x over previous
"""BitLinear158 Trainium2 kernel (per-core body + host driver).

Per core: x_shard [M_LOC, K] bf16 -> per-token int8 quant -> bf16 matmul
against host-unpacked ternary wT [K, N] -> rescale -> y [M_LOC, N] bf16.

v3 pipeline (quant AFTER transpose; no DRAM round-trip for activations):
  amax path (natural layout, per m-tile):
    x tile [128,2048]  <- scalar-ring DMA
    amax = DVE reduce(abs max); s_bf = bf16(127/amax); r = 1/s/ws (f32)
    s stored to s_dram row; per chunk: s_row load + gpsimd partition
    broadcast -> s_bc [128, cw] f32
  transpose path (independent of quant; src is the raw x input in DRAM):
    xT[kc] [128, cw] bf16 <- sync-ring xbar DMA transpose of x slice
  quant (k-layout, per (chunk, kc)):
    xq8  = DVE tensor_tensor(mult, xT, s_bc) -> int8 (RNE+saturate)
    xqbf = DVE tensor_copy(xq8) -> bf16
  weights: wT int8 [K, N] in DRAM; SWDGE (gpsimd) per-kc DMA-cast loads
    int8 -> bf16 into SBUF (first k-chunk ready ~1us after start)
  matmul:  PSUM [128m,512n] f32 += xqbf[kc][:,mi].T @ wT[:,kc,nt] (16 kc)
  rescale: y_sb = ACT activation(Copy, scale=r) -> bf16 ; y <- gpsimd DMA
"""

import sys

sys.path.insert(0, "/opt/trn_rl_repo")

from contextlib import ExitStack

import numpy as np
import ml_dtypes

import concourse.bass as bass
import concourse.tile as tile
from concourse import bacc, mybir
from concourse import bass_utils

P = 128
M_LOC = 4096      # tokens per core
K = 2048          # in features
N = 2048          # out features
KC = K // P       # 16 k-chunks
NT = M_LOC // P   # 32 m-tiles per core
# chunk sizes in m-tiles: small first chunks so the PE starts early
CHUNK_MTS = [1, 1, 2, 3, 4, 4, 4, 4, 4, 5]
assert sum(CHUNK_MTS) == NT
CHUNK_STARTS = [sum(CHUNK_MTS[:i]) for i in range(len(CHUNK_MTS))]
CHUNKS = len(CHUNK_MTS)
MAX_CHUNK_MT = max(CHUNK_MTS)
N_TILE = 512
NTN = N // N_TILE                  # 4
N_CORES = 8

BF16 = mybir.dt.bfloat16
F32 = mybir.dt.float32
I8 = mybir.dt.int8


def build_kernel():
    nc = bacc.Bacc("TRN2", target_bir_lowering=False, debug=False, num_devices=N_CORES)
    x = nc.dram_tensor("x", [M_LOC, K], BF16, kind="ExternalInput").ap()
    wTi8 = nc.dram_tensor("wTi8", [K, N], I8, kind="ExternalInput").ap()
    inv_ws = nc.dram_tensor("inv_ws", [1, 1], F32, kind="ExternalInput").ap()
    y = nc.dram_tensor("y", [M_LOC, N], BF16, kind="ExternalOutput").ap()

    x_tiled = x.rearrange("(t p) k -> t p k", p=P)
    y_tiled = y.rearrange("(t p) n -> t p n", p=P)
    wT_tiled = wTi8.rearrange("(c p) n -> p c n", p=P)

    with tile.TileContext(nc) as tc, ExitStack() as ctx:
        wbuf = ctx.enter_context(tc.tile_pool(name="wbuf", bufs=1))
        xin = ctx.enter_context(tc.tile_pool(name="xin", bufs=4))
        xT_pool = ctx.enter_context(tc.tile_pool(name="xT", bufs=2))
        xq8_pool = ctx.enter_context(tc.tile_pool(name="xq8", bufs=2))
        xqbf_pool = ctx.enter_context(tc.tile_pool(name="xqbf", bufs=2))
        stat = ctx.enter_context(tc.tile_pool(name="stat", bufs=4))
        rbuf = ctx.enter_context(tc.tile_pool(name="rbuf", bufs=1))
        srow_pool = ctx.enter_context(tc.tile_pool(name="srow", bufs=1))
        sbc_pool = ctx.enter_context(tc.tile_pool(name="sbc", bufs=2))
        yout = ctx.enter_context(tc.tile_pool(name="yout", bufs=4))
        psum = ctx.enter_context(tc.tile_pool(name="psum", bufs=8, space="PSUM"))
        dram = ctx.enter_context(tc.tile_pool(name="dram", bufs=1, space="DRAM"))

        # weights: grouped SWDGE cast loads (int8 DRAM -> bf16 SBUF); the
        # first group (kc 0-3) lands early so matmuls are never weight-gated.
        wt = wbuf.tile([P, KC, N], BF16)
        WG = 4
        for g in range(KC // WG):
            nc.gpsimd.dma_start(
                wt[:, g * WG : (g + 1) * WG, :], wT_tiled[:, g * WG : (g + 1) * WG, :]
            )

        # 1/weight_scale, broadcast to all partitions once
        iws1 = stat.tile([1, 1], F32, tag="iws1", name="iws1")
        nc.scalar.dma_start(iws1[:], inv_ws)
        iws_bc = stat.tile([P, 1], F32, tag="iws_bc", name="iws_bc")
        nc.gpsimd.partition_broadcast(iws_bc[:], iws1[:])

        r_all = rbuf.tile([P, NT], F32)
        s_dram = dram.tile([NT, P], F32, tag="s_dram", name="s_dram")
        s_cols = {}  # chunk -> [P, cmt] f32 tile of per-token scales

        # x loads for the amax path run on the scalar (ACT) HWDGE ring
        xt_tiles = {}

        def load_tile(mt):
            if mt >= NT or mt in xt_tiles:
                return
            xt = xin.tile([P, K], BF16, tag="xt", name="xt")
            nc.scalar.dma_start(xt[:], x_tiled[mt])
            xt_tiles[mt] = xt

        def chunk_of(mt):
            for c in range(CHUNKS):
                if mt < CHUNK_STARTS[c] + CHUNK_MTS[c]:
                    return c, mt - CHUNK_STARTS[c]
            raise AssertionError

        def amax_tile(mt):
            load_tile(mt)
            load_tile(mt + 1)
            load_tile(mt + 2)
            xt = xt_tiles.pop(mt)
            c, mi = chunk_of(mt)
            if c not in s_cols:
                s_cols[c] = stat.tile(
                    [P, MAX_CHUNK_MT], F32, tag="s_col", name="s_col"
                )

            amax = stat.tile([P, 1], F32, tag="amax", name="amax")
            nc.vector.tensor_reduce(
                amax[:], xt[:], axis=mybir.AxisListType.X,
                op=mybir.AluOpType.max, apply_absolute_value=True,
            )
            nc.vector.tensor_scalar_max(amax[:], amax[:], 1e-5)
            q = stat.tile([P, 1], F32, tag="q", name="q")
            nc.vector.reciprocal(q[:], amax[:])
            s_bf = stat.tile([P, 1], BF16, tag="s_bf", name="s_bf")
            nc.vector.tensor_scalar_mul(s_bf[:], q[:], 127.0)
            s_f32 = s_cols[c][:, mi : mi + 1]
            nc.vector.tensor_copy(s_f32, s_bf[:])
            r0 = stat.tile([P, 1], F32, tag="r0", name="r0")
            nc.vector.reciprocal(r0[:], s_f32)
            nc.vector.tensor_tensor(
                r_all[:, mt : mt + 1], r0[:], iws_bc[:], op=mybir.AluOpType.mult
            )

        def store_s_chunk(c):
            # one store per chunk: [P, cmt] SBUF -> s_dram[c0:c0+cmt, :]
            cmt = CHUNK_MTS[c]
            c0 = CHUNK_STARTS[c]
            nc.scalar.dma_start(
                s_dram[c0 : c0 + cmt, :].rearrange("j p -> p j"),
                s_cols.pop(c)[:, :cmt],
            )

        def srow_sbc(c):
            cmt = CHUNK_MTS[c]
            cw = cmt * P
            c0 = CHUNK_STARTS[c]
            srow = srow_pool.tile([1, MAX_CHUNK_MT * P], F32, tag="srow", name="srow")
            nc.scalar.dma_start(
                srow[0:1, :cw],
                s_dram[c0 : c0 + cmt, :].rearrange("j p -> (j p)").unsqueeze(0),
            )
            sbc = sbc_pool.tile([P, MAX_CHUNK_MT * P], F32, tag="sbc", name="sbc")
            nc.gpsimd.partition_broadcast(sbc[:, :cw], srow[0:1, :cw])
            return sbc

        def transpose_chunk(c):
            # single xbar transpose per chunk: x[m0:m0+cw, :] -> [P, KC, cw]
            # (3D dst: extra dims are logically part of the partition dim, so
            # element (p, kc, m) = x[m0+m, kc*P+p])
            cw = CHUNK_MTS[c] * P
            m0 = CHUNK_STARTS[c] * P
            tt = xT_pool.tile([P, KC, MAX_CHUNK_MT * P], BF16, tag="xT", name="xT")
            nc.sync.dma_start_transpose(tt[:, :, :cw], x[m0 : m0 + cw, :])
            return tt

        def quant_chunk(c, xT, sbc):
            cw = CHUNK_MTS[c] * P
            tiles = []
            for kc in range(KC):
                q8 = xq8_pool.tile(
                    [P, MAX_CHUNK_MT * P], I8, tag=f"xq8{kc}", name=f"xq8{kc}"
                )
                nc.vector.tensor_tensor(
                    q8[:, :cw], xT[:, kc, :cw], sbc[:, :cw], op=mybir.AluOpType.mult
                )
                qb = xqbf_pool.tile(
                    [P, MAX_CHUNK_MT * P], BF16, tag=f"xqbf{kc}", name=f"xqbf{kc}"
                )
                nc.vector.tensor_copy(qb[:, :cw], q8[:, :cw])
                tiles.append(qb)
            return tiles

        def matmul_mtile(c, mi, xqbf):
            mt = CHUNK_STARTS[c] + mi
            y_sb = yout.tile([P, N], BF16, tag="y_sb", name="y_sb")
            for nt in range(NTN):
                ps = psum.tile([P, N_TILE], F32, tag="ps", name="ps")
                for kc in range(KC):
                    nc.tensor.matmul(
                        ps[:],
                        xqbf[kc][:, mi * P : (mi + 1) * P],
                        wt[:, kc, nt * N_TILE : (nt + 1) * N_TILE],
                        start=(kc == 0),
                        stop=(kc == KC - 1),
                    )
                nc.scalar.activation(
                    y_sb[:, nt * N_TILE : (nt + 1) * N_TILE],
                    ps[:],
                    mybir.ActivationFunctionType.Copy,
                    scale=r_all[:, mt : mt + 1],
                )
            nc.gpsimd.dma_start(y_tiled[mt], y_sb[:])

        def amax_chunk(c):
            for mt in range(CHUNK_STARTS[c], CHUNK_STARTS[c] + CHUNK_MTS[c]):
                amax_tile(mt)
            store_s_chunk(c)

        # software pipeline: transposes/quant of chunk c+1 and amax of
        # chunk c+2 run during chunk c's matmuls.
        amax_chunk(0)
        amax_chunk(1)
        sbc0 = srow_sbc(0)
        xq_map = {0: quant_chunk(0, transpose_chunk(0), sbc0)}
        for c in range(CHUNKS):
            if c + 1 < CHUNKS:
                sbc = srow_sbc(c + 1)
                xq_map[c + 1] = quant_chunk(c + 1, transpose_chunk(c + 1), sbc)
            if c + 2 < CHUNKS:
                amax_chunk(c + 2)
            for mi in range(CHUNK_MTS[c]):
                matmul_mtile(c, mi, xq_map[c])
            del xq_map[c]

    nc.compile()
    return nc


def unpack_wT_i8(packed_weight: np.ndarray) -> np.ndarray:
    planes = [((packed_weight >> (2 * i)) & 3) for i in range(4)]
    w = np.concatenate(planes, axis=0).astype(np.int8) - 1  # [N, K] ternary
    return np.ascontiguousarray(w.T)  # [K, N] int8


_CACHE = {}


def run(x: np.ndarray, packed_weight: np.ndarray, weight_scale: np.ndarray,
        trace: bool = False, tmpdir=None):
    """x: [B, S, K] bf16 -> y [B, S, N] bf16 (full, unsharded)."""
    if "nc" not in _CACHE:
        _CACHE["nc"] = build_kernel()
    nc = _CACHE["nc"]

    B, S, D = x.shape
    M = B * S
    assert M == M_LOC * N_CORES and D == K
    wT = unpack_wT_i8(packed_weight)
    iws = np.asarray(
        1.0 / np.float32(np.asarray(weight_scale).reshape(-1)[0]), dtype=np.float32
    ).reshape(1, 1)
    shards = np.ascontiguousarray(np.asarray(x).reshape(N_CORES, M_LOC, K))
    in_maps = [
        {"x": shards[i], "wTi8": wT, "inv_ws": iws} for i in range(N_CORES)
    ]
    res = bass_utils.run_bass_kernel_spmd(
        nc, in_maps, core_ids=list(range(N_CORES)), trace=trace, tmpdir=tmpdir
    )
    y = np.stack([res.results[i]["y"] for i in range(N_CORES)], axis=0)
    return y.reshape(B, S, N), res


def kernel(x, packed_weight, weight_scale):
    """Harness entrypoint: FULL inputs -> FULL output.

    x: [4, 8192, 2048] bf16; packed_weight: [512, 2048] uint8;
    weight_scale: [1] bf16.  Returns [4, 8192, 2048] bf16.
    Sharding: data-parallel over tokens across the 8 NeuronCores;
    the (host-unpacked) ternary weight is replicated.
    """
    x = np.asarray(x)
    packed_weight = np.asarray(packed_weight)
    weight_scale = np.asarray(weight_scale)
    y, _ = run(x, packed_weight, weight_scale)
    return y


# revision 10
# speedup vs baseline: 1.4345x; 1.0376x over previous
"""BitLinear158 Trainium2 kernel (per-core body + host driver).

Per core: x_shard [M_LOC, K] bf16 -> per-token int8 quant -> bf16 matmul
against host-unpacked ternary wT [K, N] -> rescale -> y [M_LOC, N] bf16.

v5 pipeline (quant AFTER transpose; no DRAM round-trip for activations):
  amax path (DVE; natural layout, x loaded in 2-m-tile pairs on
    the scalar HWDGE ring): amax -> s_bf = bf16(127/amax) -> s_col;
    r = (1/s)/ws -> r_all.  s_col stored per chunk to s_dram (SWDGE),
    re-read as a row (SWDGE) and partition-broadcast (gpsimd op).
  transpose path (independent of quant; src is the raw x input in DRAM):
    two xbar DMA transposes per chunk on the sync HWDGE ring, each
    [P, KC/2, cw] (3D dst: (p, kc, m) = x[m0+m, kc*P+p]).
  quant (DVE only, per (chunk, kc)):
    xq8  = tensor_tensor(mult, xT, s_bc) -> int8 (RNE+saturate)
    xqbf = tensor_copy(xq8) -> bf16
  weights: wT int8 [K, N] in DRAM; grouped SWDGE DMA-cast loads
    int8 -> bf16 into SBUF (first group ready early).
  matmul:  PSUM [128m,512n] f32 += xqbf[kc][:,mi].T @ wT[:,kc,nt] (16 kc)
  rescale: y_sb = ACT activation(Copy, scale=r) -> bf16 ; y <- gpsimd DMA
"""

import sys

sys.path.insert(0, "/opt/trn_rl_repo")

from contextlib import ExitStack

import numpy as np
import ml_dtypes

import concourse.bass as bass
import concourse.tile as tile
from concourse import bacc, mybir
from concourse import bass_utils

P = 128
M_LOC = 4096      # tokens per core
K = 2048          # in features
N = 2048          # out features
KC = K // P       # 16 k-chunks
KCH = KC // 2     # k-chunks per transpose piece
NT = M_LOC // P   # 32 m-tiles per core
# chunk sizes in m-tiles: small first chunks so the PE starts early
CHUNK_MTS = [1, 1, 2, 3, 4, 4, 4, 4, 4, 5]
assert sum(CHUNK_MTS) == NT
CHUNK_STARTS = [sum(CHUNK_MTS[:i]) for i in range(len(CHUNK_MTS))]
CHUNKS = len(CHUNK_MTS)
MAX_CHUNK_MT = max(CHUNK_MTS)
N_TILE = 512
NTN = N // N_TILE                  # 4
N_CORES = 8

BF16 = mybir.dt.bfloat16
F32 = mybir.dt.float32
I8 = mybir.dt.int8


def build_kernel():
    nc = bacc.Bacc("TRN2", target_bir_lowering=False, debug=False, num_devices=N_CORES)
    x = nc.dram_tensor("x", [M_LOC, K], BF16, kind="ExternalInput").ap()
    wTi8 = nc.dram_tensor("wTi8", [K, N], I8, kind="ExternalInput").ap()
    inv_ws = nc.dram_tensor("inv_ws", [1, 1], F32, kind="ExternalInput").ap()
    y = nc.dram_tensor("y", [M_LOC, N], BF16, kind="ExternalOutput").ap()

    x_pairs = x.rearrange("(j t p) k -> j p t k", t=2, p=P)
    y_tiled = y.rearrange("(t p) n -> t p n", p=P)
    wT_tiled = wTi8.rearrange("(c p) n -> p c n", p=P)

    with tile.TileContext(nc) as tc, ExitStack() as ctx:
        wbuf = ctx.enter_context(tc.tile_pool(name="wbuf", bufs=1))
        xin = ctx.enter_context(tc.tile_pool(name="xin", bufs=3))
        xT_pool = ctx.enter_context(tc.tile_pool(name="xT", bufs=2))
        xq8_pool = ctx.enter_context(tc.tile_pool(name="xq8", bufs=4))
        xqbf_pool = ctx.enter_context(tc.tile_pool(name="xqbf", bufs=2))
        stat = ctx.enter_context(tc.tile_pool(name="stat", bufs=4))
        rbuf = ctx.enter_context(tc.tile_pool(name="rbuf", bufs=1))
        srow_pool = ctx.enter_context(tc.tile_pool(name="srow", bufs=2))
        sbc_pool = ctx.enter_context(tc.tile_pool(name="sbc", bufs=2))
        yout = ctx.enter_context(tc.tile_pool(name="yout", bufs=3))
        psum = ctx.enter_context(tc.tile_pool(name="psum", bufs=8, space="PSUM"))
        dram = ctx.enter_context(tc.tile_pool(name="dram", bufs=1, space="DRAM"))

        r_all = rbuf.tile([P, NT], F32)
        s_dram = dram.tile([NT, P], F32, tag="s_dram", name="s_dram")
        s_cols = {}  # chunk -> [P, cmt] f32 tile of per-token scales

        # ---- prologue-critical DMAs first: chunk-0 transposes + x pair 0
        def transpose_chunk(c):
            # two xbar transposes per chunk: x[m0:m0+cw, half] -> [P, KCH, cw]
            cw = CHUNK_MTS[c] * P
            m0 = CHUNK_STARTS[c] * P
            tiles = []
            for h in range(2):
                tt = xT_pool.tile(
                    [P, KCH, MAX_CHUNK_MT * P], BF16, tag=f"xT{h}", name=f"xT{h}"
                )
                nc.sync.dma_start_transpose(
                    tt[:, :, :cw], x[m0 : m0 + cw, h * (K // 2) : (h + 1) * (K // 2)]
                )
                tiles.append(tt)
            return tiles

        xt_tiles = {}

        def load_pair(j):
            if j >= NT // 2 or j in xt_tiles:
                return
            xt = xin.tile([P, 2, K], BF16, tag="xt", name="xt")
            nc.scalar.dma_start(xt[:], x_pairs[j])
            xt_tiles[j] = xt

        t0 = transpose_chunk(0)
        load_pair(0)

        # weights: grouped SWDGE cast loads (int8 DRAM -> bf16 SBUF)
        wt = wbuf.tile([P, KC, N], BF16)
        WG = 4
        for g in range(KC // WG):
            nc.gpsimd.dma_start(
                wt[:, g * WG : (g + 1) * WG, :], wT_tiled[:, g * WG : (g + 1) * WG, :]
            )

        # 1/weight_scale, broadcast to all partitions once
        iws1 = stat.tile([1, 1], F32, tag="iws1", name="iws1")
        nc.gpsimd.dma_start(iws1[:], inv_ws)
        iws_bc = stat.tile([P, 1], F32, tag="iws_bc", name="iws_bc")
        nc.gpsimd.partition_broadcast(iws_bc[:], iws1[:])

        def chunk_of(mt):
            for c in range(CHUNKS):
                if mt < CHUNK_STARTS[c] + CHUNK_MTS[c]:
                    return c, mt - CHUNK_STARTS[c]
            raise AssertionError

        def amax_tile(mt):
            # whole scale chain runs on gpsimd; DVE stays quant-only
            load_pair(mt // 2)
            load_pair(mt // 2 + 1)
            xt = xt_tiles[mt // 2]
            if mt % 2 == 1:
                del xt_tiles[mt // 2]
            c, mi = chunk_of(mt)
            if c not in s_cols:
                s_cols[c] = stat.tile(
                    [P, MAX_CHUNK_MT], F32, tag="s_col", name="s_col"
                )

            amax = stat.tile([P, 1], F32, tag="amax", name="amax")
            nc.vector.tensor_reduce(
                amax[:], xt[:, mt % 2, :], axis=mybir.AxisListType.X,
                op=mybir.AluOpType.max, apply_absolute_value=True,
            )
            nc.vector.tensor_scalar_max(amax[:], amax[:], 1e-5)
            q = stat.tile([P, 1], F32, tag="q", name="q")
            nc.vector.reciprocal(q[:], amax[:])
            s_bf = stat.tile([P, 1], BF16, tag="s_bf", name="s_bf")
            nc.vector.tensor_scalar_mul(s_bf[:], q[:], 127.0)
            s_f32 = s_cols[c][:, mi : mi + 1]
            nc.vector.tensor_copy(s_f32, s_bf[:])
            r0 = stat.tile([P, 1], F32, tag="r0", name="r0")
            nc.vector.reciprocal(r0[:], s_f32)
            nc.vector.tensor_tensor(
                r_all[:, mt : mt + 1], r0[:], iws_bc[:], op=mybir.AluOpType.mult
            )

        def store_s_chunk(c):
            cmt = CHUNK_MTS[c]
            c0 = CHUNK_STARTS[c]
            nc.gpsimd.dma_start(
                s_dram[c0 : c0 + cmt, :].rearrange("j p -> p j"),
                s_cols.pop(c)[:, :cmt],
            )

        def srow_sbc(c):
            cmt = CHUNK_MTS[c]
            cw = cmt * P
            c0 = CHUNK_STARTS[c]
            srow = srow_pool.tile([1, MAX_CHUNK_MT * P], F32, tag="srow", name="srow")
            nc.gpsimd.dma_start(
                srow[0:1, :cw],
                s_dram[c0 : c0 + cmt, :].rearrange("j p -> (j p)").unsqueeze(0),
            )
            sbc = sbc_pool.tile([P, MAX_CHUNK_MT * P], F32, tag="sbc", name="sbc")
            nc.gpsimd.partition_broadcast(sbc[:, :cw], srow[0:1, :cw])
            return sbc

        def quant_chunk(c, xT, sbc):
            cw = CHUNK_MTS[c] * P
            tiles = []
            for kc in range(KC):
                src = xT[kc // KCH][:, kc % KCH, :cw]
                q8 = xq8_pool.tile(
                    [P, MAX_CHUNK_MT * P], I8, tag="xq8", name="xq8"
                )
                nc.vector.tensor_tensor(
                    q8[:, :cw], src, sbc[:, :cw], op=mybir.AluOpType.mult
                )
                qb = xqbf_pool.tile(
                    [P, MAX_CHUNK_MT * P], BF16, tag=f"xqbf{kc}", name=f"xqbf{kc}"
                )
                nc.vector.tensor_copy(qb[:, :cw], q8[:, :cw])
                tiles.append(qb)
            return tiles

        def matmul_mtile(c, mi, xqbf):
            mt = CHUNK_STARTS[c] + mi
            y_sb = yout.tile([P, N], BF16, tag="y_sb", name="y_sb")
            for nt in range(NTN):
                ps = psum.tile([P, N_TILE], F32, tag="ps", name="ps")
                for kc in range(KC):
                    nc.tensor.matmul(
                        ps[:],
                        xqbf[kc][:, mi * P : (mi + 1) * P],
                        wt[:, kc, nt * N_TILE : (nt + 1) * N_TILE],
                        start=(kc == 0),
                        stop=(kc == KC - 1),
                    )
                nc.scalar.activation(
                    y_sb[:, nt * N_TILE : (nt + 1) * N_TILE],
                    ps[:],
                    mybir.ActivationFunctionType.Copy,
                    scale=r_all[:, mt : mt + 1],
                )
            nc.gpsimd.dma_start(y_tiled[mt], y_sb[:])

        def amax_chunk(c):
            for mt in range(CHUNK_STARTS[c], CHUNK_STARTS[c] + CHUNK_MTS[c]):
                amax_tile(mt)
            store_s_chunk(c)

        # software pipeline: transposes/quant of chunk c+1 and amax of
        # chunk c+3 run during chunk c's matmuls.
        amax_chunk(0)
        amax_chunk(1)
        amax_chunk(2)
        sbc0 = srow_sbc(0)
        xq_map = {0: quant_chunk(0, t0, sbc0)}
        for c in range(CHUNKS):
            if c + 1 < CHUNKS:
                tc1 = transpose_chunk(c + 1)
                sbc = srow_sbc(c + 1)
                xq_map[c + 1] = quant_chunk(c + 1, tc1, sbc)
            if c + 3 < CHUNKS:
                amax_chunk(c + 3)
            for mi in range(CHUNK_MTS[c]):
                matmul_mtile(c, mi, xq_map[c])
            del xq_map[c]

    nc.compile()
    return nc


def unpack_wT_i8(packed_weight: np.ndarray) -> np.ndarray:
    planes = [((packed_weight >> (2 * i)) & 3) for i in range(4)]
    w = np.concatenate(planes, axis=0).astype(np.int8) - 1  # [N, K] ternary
    return np.ascontiguousarray(w.T)  # [K, N] int8


_CACHE = {}


def run(x: np.ndarray, packed_weight: np.ndarray, weight_scale: np.ndarray,
        trace: bool = False, tmpdir=None):
    """x: [B, S, K] bf16 -> y [B, S, N] bf16 (full, unsharded)."""
    if "nc" not in _CACHE:
        _CACHE["nc"] = build_kernel()
    nc = _CACHE["nc"]

    B, S, D = x.shape
    M = B * S
    assert M == M_LOC * N_CORES and D == K
    wT = unpack_wT_i8(packed_weight)
    iws = np.asarray(
        1.0 / np.float32(np.asarray(weight_scale).reshape(-1)[0]), dtype=np.float32
    ).reshape(1, 1)
    shards = np.ascontiguousarray(np.asarray(x).reshape(N_CORES, M_LOC, K))
    in_maps = [
        {"x": shards[i], "wTi8": wT, "inv_ws": iws} for i in range(N_CORES)
    ]
    res = bass_utils.run_bass_kernel_spmd(
        nc, in_maps, core_ids=list(range(N_CORES)), trace=trace, tmpdir=tmpdir
    )
    y = np.stack([res.results[i]["y"] for i in range(N_CORES)], axis=0)
    return y.reshape(B, S, N), res


def kernel(x, packed_weight, weight_scale):
    """Harness entrypoint: FULL inputs -> FULL output.

    x: [4, 8192, 2048] bf16; packed_weight: [512, 2048] uint8;
    weight_scale: [1] bf16.  Returns [4, 8192, 2048] bf16.
    Sharding: data-parallel over tokens across the 8 NeuronCores;
    the (host-unpacked) ternary weight is replicated.
    """
    x = np.asarray(x)
    packed_weight = np.asarray(packed_weight)
    weight_scale = np.asarray(weight_scale)
    y, _ = run(x, packed_weight, weight_scale)
    return y


# revision 11
# speedup vs baseline: 1.5367x; 1.0712x over previous
"""BitLinear158 Trainium2 kernel (per-core body + host driver).

Per core: x_shard [M_LOC, K] bf16 -> per-token int8 quant -> bf16 matmul
against host-unpacked ternary wT [K, N] -> rescale -> y [M_LOC, N] bf16.

v5 pipeline (quant AFTER transpose; no DRAM round-trip for activations):
  amax path (DVE; natural layout, x loaded in 2-m-tile pairs on
    the scalar HWDGE ring): amax -> s_bf = bf16(127/amax) -> s_col;
    r = (1/s)/ws -> r_all.  s_col stored per chunk to s_dram (SWDGE),
    re-read as a row (SWDGE) and partition-broadcast (gpsimd op).
  transpose path (independent of quant; src is the raw x input in DRAM):
    two xbar DMA transposes per chunk on the sync HWDGE ring, each
    [P, KC/2, cw] (3D dst: (p, kc, m) = x[m0+m, kc*P+p]).
  quant (DVE only, per (chunk, kc)):
    xq8  = tensor_tensor(mult, xT, s_bc) -> int8 (RNE+saturate)
    xqbf = tensor_copy(xq8) -> bf16
  weights: wT fp8e4 [K, N] in DRAM (ternary is fp8-exact); grouped HWDGE
    loads on the scalar ring; matmul moving operand reads fp8 directly.
  matmul:  PSUM [128m,512n] f32 += xqbf[kc][:,mi].T @ wT[:,kc,nt] (16 kc)
  rescale: y_sb = ACT activation(Copy, scale=r) -> bf16 ; y <- gpsimd DMA
"""

import sys

sys.path.insert(0, "/opt/trn_rl_repo")

from contextlib import ExitStack

import numpy as np
import ml_dtypes

import concourse.bass as bass
import concourse.tile as tile
from concourse import bacc, mybir
from concourse import bass_utils

P = 128
M_LOC = 4096      # tokens per core
K = 2048          # in features
N = 2048          # out features
KC = K // P       # 16 k-chunks
KCH = KC // 2     # k-chunks per transpose piece
NT = M_LOC // P   # 32 m-tiles per core
# chunk sizes in m-tiles: small first chunks so the PE starts early
CHUNK_MTS = [1, 1, 2, 3, 4, 4, 4, 4, 4, 5]
assert sum(CHUNK_MTS) == NT
CHUNK_STARTS = [sum(CHUNK_MTS[:i]) for i in range(len(CHUNK_MTS))]
CHUNKS = len(CHUNK_MTS)
MAX_CHUNK_MT = max(CHUNK_MTS)
N_TILE = 512
NTN = N // N_TILE                  # 4
N_CORES = 8

BF16 = mybir.dt.bfloat16
F32 = mybir.dt.float32
I8 = mybir.dt.int8
F8 = mybir.dt.float8e4


def build_kernel():
    nc = bacc.Bacc("TRN2", target_bir_lowering=False, debug=False, num_devices=N_CORES)
    x = nc.dram_tensor("x", [M_LOC, K], BF16, kind="ExternalInput").ap()
    wTf8 = nc.dram_tensor("wTf8", [K, N], F8, kind="ExternalInput").ap()
    inv_ws = nc.dram_tensor("inv_ws", [1, 1], F32, kind="ExternalInput").ap()
    y = nc.dram_tensor("y", [M_LOC, N], BF16, kind="ExternalOutput").ap()

    x_pairs = x.rearrange("(j t p) k -> j p t k", t=2, p=P)
    y_tiled = y.rearrange("(t p) n -> t p n", p=P)
    wT_tiled = wTf8.rearrange("(c p) n -> p c n", p=P)

    with tile.TileContext(nc) as tc, ExitStack() as ctx:
        wbuf = ctx.enter_context(tc.tile_pool(name="wbuf", bufs=1))
        xin = ctx.enter_context(tc.tile_pool(name="xin", bufs=3))
        xT_pool = ctx.enter_context(tc.tile_pool(name="xT", bufs=2))
        xq8_pool = ctx.enter_context(tc.tile_pool(name="xq8", bufs=4))
        xqbf_pool = ctx.enter_context(tc.tile_pool(name="xqbf", bufs=2))
        stat = ctx.enter_context(tc.tile_pool(name="stat", bufs=4))
        rbuf = ctx.enter_context(tc.tile_pool(name="rbuf", bufs=1))
        srow_pool = ctx.enter_context(tc.tile_pool(name="srow", bufs=2))
        sbc_pool = ctx.enter_context(tc.tile_pool(name="sbc", bufs=2))
        yout = ctx.enter_context(tc.tile_pool(name="yout", bufs=3))
        psum = ctx.enter_context(tc.tile_pool(name="psum", bufs=8, space="PSUM"))
        dram = ctx.enter_context(tc.tile_pool(name="dram", bufs=1, space="DRAM"))

        r_all = rbuf.tile([P, NT], F32)
        s_dram = dram.tile([NT, P], F32, tag="s_dram", name="s_dram")
        s_cols = {}  # chunk -> [P, cmt] f32 tile of per-token scales

        # ---- prologue-critical DMAs first: chunk-0 transposes + x pair 0
        def transpose_chunk(c):
            # two xbar transposes per chunk: x[m0:m0+cw, half] -> [P, KCH, cw]
            cw = CHUNK_MTS[c] * P
            m0 = CHUNK_STARTS[c] * P
            tiles = []
            for h in range(2):
                tt = xT_pool.tile(
                    [P, KCH, MAX_CHUNK_MT * P], BF16, tag=f"xT{h}", name=f"xT{h}"
                )
                nc.sync.dma_start_transpose(
                    tt[:, :, :cw], x[m0 : m0 + cw, h * (K // 2) : (h + 1) * (K // 2)]
                )
                tiles.append(tt)
            return tiles

        xt_tiles = {}

        def load_pair(j):
            if j >= NT // 2 or j in xt_tiles:
                return
            xt = xin.tile([P, 2, K], BF16, tag="xt", name="xt")
            nc.scalar.dma_start(xt[:], x_pairs[j])
            xt_tiles[j] = xt

        t0 = transpose_chunk(0)
        load_pair(0)

        # 1/weight_scale first on gpsimd (tiny; must not queue behind bulk)
        iws1 = stat.tile([1, 1], F32, tag="iws1", name="iws1")
        nc.gpsimd.dma_start(iws1[:], inv_ws)
        iws_bc = stat.tile([P, 1], F32, tag="iws_bc", name="iws_bc")
        nc.gpsimd.partition_broadcast(iws_bc[:], iws1[:])

        # weights: ternary is exact in fp8e4; mixed bf16(stationary) x fp8
        # (moving) matmul runs at full rate, so no on-chip cast is needed.
        # Grouped HWDGE loads on the scalar ring (first group lands early).
        wt = wbuf.tile([P, KC, N], F8)
        WG = 4
        for g in range(KC // WG):
            nc.scalar.dma_start(
                wt[:, g * WG : (g + 1) * WG, :], wT_tiled[:, g * WG : (g + 1) * WG, :]
            )

        def chunk_of(mt):
            for c in range(CHUNKS):
                if mt < CHUNK_STARTS[c] + CHUNK_MTS[c]:
                    return c, mt - CHUNK_STARTS[c]
            raise AssertionError

        def amax_tile(mt):
            # whole scale chain runs on gpsimd; DVE stays quant-only
            load_pair(mt // 2)
            load_pair(mt // 2 + 1)
            xt = xt_tiles[mt // 2]
            if mt % 2 == 1:
                del xt_tiles[mt // 2]
            c, mi = chunk_of(mt)
            if c not in s_cols:
                s_cols[c] = stat.tile(
                    [P, MAX_CHUNK_MT], F32, tag="s_col", name="s_col"
                )

            amax = stat.tile([P, 1], F32, tag="amax", name="amax")
            nc.vector.tensor_reduce(
                amax[:], xt[:, mt % 2, :], axis=mybir.AxisListType.X,
                op=mybir.AluOpType.max, apply_absolute_value=True,
            )
            nc.vector.tensor_scalar_max(amax[:], amax[:], 1e-5)
            q = stat.tile([P, 1], F32, tag="q", name="q")
            nc.vector.reciprocal(q[:], amax[:])
            s_bf = stat.tile([P, 1], BF16, tag="s_bf", name="s_bf")
            nc.vector.tensor_scalar_mul(s_bf[:], q[:], 127.0)
            s_f32 = s_cols[c][:, mi : mi + 1]
            nc.vector.tensor_copy(s_f32, s_bf[:])
            r0 = stat.tile([P, 1], F32, tag="r0", name="r0")
            nc.vector.reciprocal(r0[:], s_f32)
            nc.vector.tensor_tensor(
                r_all[:, mt : mt + 1], r0[:], iws_bc[:], op=mybir.AluOpType.mult
            )

        def store_s_chunk(c):
            cmt = CHUNK_MTS[c]
            c0 = CHUNK_STARTS[c]
            nc.gpsimd.dma_start(
                s_dram[c0 : c0 + cmt, :].rearrange("j p -> p j"),
                s_cols.pop(c)[:, :cmt],
            )

        def srow_sbc(c):
            cmt = CHUNK_MTS[c]
            cw = cmt * P
            c0 = CHUNK_STARTS[c]
            srow = srow_pool.tile([1, MAX_CHUNK_MT * P], F32, tag="srow", name="srow")
            nc.gpsimd.dma_start(
                srow[0:1, :cw],
                s_dram[c0 : c0 + cmt, :].rearrange("j p -> (j p)").unsqueeze(0),
            )
            sbc = sbc_pool.tile([P, MAX_CHUNK_MT * P], F32, tag="sbc", name="sbc")
            nc.gpsimd.partition_broadcast(sbc[:, :cw], srow[0:1, :cw])
            return sbc

        def quant_chunk(c, xT, sbc):
            cw = CHUNK_MTS[c] * P
            tiles = []
            for kc in range(KC):
                src = xT[kc // KCH][:, kc % KCH, :cw]
                q8 = xq8_pool.tile(
                    [P, MAX_CHUNK_MT * P], I8, tag="xq8", name="xq8"
                )
                nc.vector.tensor_tensor(
                    q8[:, :cw], src, sbc[:, :cw], op=mybir.AluOpType.mult
                )
                qb = xqbf_pool.tile(
                    [P, MAX_CHUNK_MT * P], BF16, tag=f"xqbf{kc}", name=f"xqbf{kc}"
                )
                nc.vector.tensor_copy(qb[:, :cw], q8[:, :cw])
                tiles.append(qb)
            return tiles

        def matmul_mtile(c, mi, xqbf):
            mt = CHUNK_STARTS[c] + mi
            y_sb = yout.tile([P, N], BF16, tag="y_sb", name="y_sb")
            for nt in range(NTN):
                ps = psum.tile([P, N_TILE], F32, tag="ps", name="ps")
                for kc in range(KC):
                    nc.tensor.matmul(
                        ps[:],
                        xqbf[kc][:, mi * P : (mi + 1) * P],
                        wt[:, kc, nt * N_TILE : (nt + 1) * N_TILE],
                        start=(kc == 0),
                        stop=(kc == KC - 1),
                    )
                nc.scalar.activation(
                    y_sb[:, nt * N_TILE : (nt + 1) * N_TILE],
                    ps[:],
                    mybir.ActivationFunctionType.Copy,
                    scale=r_all[:, mt : mt + 1],
                )
            nc.gpsimd.dma_start(y_tiled[mt], y_sb[:])

        def amax_chunk(c):
            for mt in range(CHUNK_STARTS[c], CHUNK_STARTS[c] + CHUNK_MTS[c]):
                amax_tile(mt)
            store_s_chunk(c)

        # software pipeline: transposes/quant of chunk c+1 and amax of
        # chunk c+3 run during chunk c's matmuls.
        amax_chunk(0)
        amax_chunk(1)
        sbc0 = srow_sbc(0)
        xq_map = {0: quant_chunk(0, t0, sbc0)}
        amax_chunk(2)
        for c in range(CHUNKS):
            if c + 1 < CHUNKS:
                tc1 = transpose_chunk(c + 1)
                sbc = srow_sbc(c + 1)
                xq_map[c + 1] = quant_chunk(c + 1, tc1, sbc)
            if c + 3 < CHUNKS:
                amax_chunk(c + 3)
            for mi in range(CHUNK_MTS[c]):
                matmul_mtile(c, mi, xq_map[c])
            del xq_map[c]

    nc.compile()
    return nc


def unpack_wT_f8(packed_weight: np.ndarray) -> np.ndarray:
    planes = [((packed_weight >> (2 * i)) & 3) for i in range(4)]
    w = np.concatenate(planes, axis=0).astype(np.float32) - 1.0  # [N, K] ternary
    return np.ascontiguousarray(w.T).astype(ml_dtypes.float8_e4m3)  # [K, N] fp8


_CACHE = {}


def run(x: np.ndarray, packed_weight: np.ndarray, weight_scale: np.ndarray,
        trace: bool = False, tmpdir=None):
    """x: [B, S, K] bf16 -> y [B, S, N] bf16 (full, unsharded)."""
    if "nc" not in _CACHE:
        _CACHE["nc"] = build_kernel()
    nc = _CACHE["nc"]

    B, S, D = x.shape
    M = B * S
    assert M == M_LOC * N_CORES and D == K
    wT = unpack_wT_f8(packed_weight)
    iws = np.asarray(
        1.0 / np.float32(np.asarray(weight_scale).reshape(-1)[0]), dtype=np.float32
    ).reshape(1, 1)
    shards = np.ascontiguousarray(np.asarray(x).reshape(N_CORES, M_LOC, K))
    in_maps = [
        {"x": shards[i], "wTf8": wT, "inv_ws": iws} for i in range(N_CORES)
    ]
    res = bass_utils.run_bass_kernel_spmd(
        nc, in_maps, core_ids=list(range(N_CORES)), trace=trace, tmpdir=tmpdir
    )
    y = np.stack([res.results[i]["y"] for i in range(N_CORES)], axis=0)
    return y.reshape(B, S, N), res


def kernel(x, packed_weight, weight_scale):
    """Harness entrypoint: FULL inputs -> FULL output.

    x: [4, 8192, 2048] bf16; packed_weight: [512, 2048] uint8;
    weight_scale: [1] bf16.  Returns [4, 8192, 2048] bf16.
    Sharding: data-parallel over tokens across the 8 NeuronCores;
    the (host-unpacked) ternary weight is replicated.
    """
    x = np.asarray(x)
    packed_weight = np.asarray(packed_weight)
    weight_scale = np.asarray(weight_scale)
    y, _ = run(x, packed_weight, weight_scale)
    return y


# revision 12
# speedup vs baseline: 1.5909x; 1.0353x over previous
"""BitLinear158 Trainium2 kernel (per-core body + host driver).

Per core: x_shard [M_LOC, K] bf16 -> per-token int8 quant -> bf16 matmul
against host-unpacked ternary wT [K, N] -> rescale -> y [M_LOC, N] bf16.

v5 pipeline (quant AFTER transpose; no DRAM round-trip for activations):
  amax path (DVE; natural layout, x loaded in 2-m-tile pairs on
    the scalar HWDGE ring): amax -> s_bf = bf16(127/amax) -> s_col;
    r = (1/s)/ws -> r_all.  s_col stored per chunk to s_dram (SWDGE),
    re-read as a row (SWDGE) and partition-broadcast (gpsimd op).
  transpose path (independent of quant; src is the raw x input in DRAM):
    two xbar DMA transposes per chunk on the sync HWDGE ring, each
    [P, KC/2, cw] (3D dst: (p, kc, m) = x[m0+m, kc*P+p]).
  quant (DVE only, per (chunk, kc)):
    xq8  = tensor_tensor(mult, xT, s_bc) -> int8 (RNE+saturate)
    xqbf = tensor_copy(xq8) -> bf16
  weights: wT fp8e4 [K, N] in DRAM (ternary is fp8-exact); grouped HWDGE
    loads on the scalar ring; matmul moving operand reads fp8 directly.
  matmul:  PSUM [128m,512n] f32 += xqbf[kc][:,mi].T @ wT[:,kc,nt] (16 kc)
  rescale: y_sb = ACT activation(Copy, scale=r) -> bf16 ; y <- gpsimd DMA
"""

import sys

sys.path.insert(0, "/opt/trn_rl_repo")

from contextlib import ExitStack

import numpy as np
import ml_dtypes

import concourse.bass as bass
import concourse.tile as tile
from concourse import bacc, mybir
from concourse import bass_utils

P = 128
M_LOC = 4096      # tokens per core
K = 2048          # in features
N = 2048          # out features
KC = K // P       # 16 k-chunks
KCH = KC // 2     # k-chunks per transpose piece
NT = M_LOC // P   # 32 m-tiles per core
# chunk sizes in m-tiles: small first chunks so the PE starts early
CHUNK_MTS = [1, 1, 2, 3, 4, 4, 4, 4, 4, 5]
assert sum(CHUNK_MTS) == NT
CHUNK_STARTS = [sum(CHUNK_MTS[:i]) for i in range(len(CHUNK_MTS))]
CHUNKS = len(CHUNK_MTS)
MAX_CHUNK_MT = max(CHUNK_MTS)
N_TILE = 512
NTN = N // N_TILE                  # 4
N_CORES = 8

BF16 = mybir.dt.bfloat16
F32 = mybir.dt.float32
I8 = mybir.dt.int8
F8 = mybir.dt.float8e4


def build_kernel():
    nc = bacc.Bacc("TRN2", target_bir_lowering=False, debug=False, num_devices=N_CORES)
    x = nc.dram_tensor("x", [M_LOC, K], BF16, kind="ExternalInput").ap()
    wTf8 = nc.dram_tensor("wTf8", [K, N], F8, kind="ExternalInput").ap()
    inv_ws = nc.dram_tensor("inv_ws", [1, 1], F32, kind="ExternalInput").ap()
    y = nc.dram_tensor("y", [M_LOC, N], BF16, kind="ExternalOutput").ap()

    x_pairs = x.rearrange("(j t p) k -> j p t k", t=2, p=P)
    y_tiled = y.rearrange("(t p) n -> t p n", p=P)
    wT_tiled = wTf8.rearrange("(c p) n -> p c n", p=P)

    with tile.TileContext(nc) as tc, ExitStack() as ctx:
        wbuf = ctx.enter_context(tc.tile_pool(name="wbuf", bufs=1))
        xin = ctx.enter_context(tc.tile_pool(name="xin", bufs=3))
        xT_pool = ctx.enter_context(tc.tile_pool(name="xT", bufs=2))
        xq8_pool = ctx.enter_context(tc.tile_pool(name="xq8", bufs=4))
        xqbf_pool = ctx.enter_context(tc.tile_pool(name="xqbf", bufs=2))
        stat = ctx.enter_context(tc.tile_pool(name="stat", bufs=4))
        rbuf = ctx.enter_context(tc.tile_pool(name="rbuf", bufs=1))
        srow_pool = ctx.enter_context(tc.tile_pool(name="srow", bufs=2))
        sbc_pool = ctx.enter_context(tc.tile_pool(name="sbc", bufs=4))
        yout = ctx.enter_context(tc.tile_pool(name="yout", bufs=3))
        psum = ctx.enter_context(tc.tile_pool(name="psum", bufs=8, space="PSUM"))
        dram = ctx.enter_context(tc.tile_pool(name="dram", bufs=1, space="DRAM"))

        r_all = rbuf.tile([P, NT], F32)
        s_dram = dram.tile([NT, P], F32, tag="s_dram", name="s_dram")
        s_cols = {}  # chunk -> [P, cmt] f32 tile of per-token scales

        # ---- prologue-critical DMAs first: chunk-0 transposes + x pair 0
        def transpose_chunk(c):
            # two xbar transposes per chunk: x[m0:m0+cw, half] -> [P, KCH, cw]
            cw = CHUNK_MTS[c] * P
            m0 = CHUNK_STARTS[c] * P
            tiles = []
            for h in range(2):
                tt = xT_pool.tile(
                    [P, KCH, MAX_CHUNK_MT * P], BF16, tag=f"xT{h}", name=f"xT{h}"
                )
                nc.sync.dma_start_transpose(
                    tt[:, :, :cw], x[m0 : m0 + cw, h * (K // 2) : (h + 1) * (K // 2)]
                )
                tiles.append(tt)
            return tiles

        xt_tiles = {}

        def load_pair(j):
            if j >= NT // 2 or j in xt_tiles:
                return
            xt = xin.tile([P, 2, K], BF16, tag="xt", name="xt")
            nc.scalar.dma_start(xt[:], x_pairs[j])
            xt_tiles[j] = xt

        t0 = transpose_chunk(0)
        load_pair(0)

        # 1/weight_scale first on gpsimd (tiny; must not queue behind bulk)
        iws1 = stat.tile([1, 1], F32, tag="iws1", name="iws1")
        nc.gpsimd.dma_start(iws1[:], inv_ws)
        iws_bc = stat.tile([P, 1], F32, tag="iws_bc", name="iws_bc")
        nc.gpsimd.partition_broadcast(iws_bc[:], iws1[:])

        # weights: ternary is exact in fp8e4; mixed bf16(stationary) x fp8
        # (moving) matmul runs at full rate, so no on-chip cast is needed.
        # Grouped HWDGE loads on the scalar ring (first group lands early).
        wt = wbuf.tile([P, KC, N], F8)
        WG = 4
        for g in range(KC // WG):
            nc.scalar.dma_start(
                wt[:, g * WG : (g + 1) * WG, :], wT_tiled[:, g * WG : (g + 1) * WG, :]
            )

        def chunk_of(mt):
            for c in range(CHUNKS):
                if mt < CHUNK_STARTS[c] + CHUNK_MTS[c]:
                    return c, mt - CHUNK_STARTS[c]
            raise AssertionError

        def amax_tile(mt):
            # whole scale chain runs on gpsimd; DVE stays quant-only
            load_pair(mt // 2)
            load_pair(mt // 2 + 1)
            xt = xt_tiles[mt // 2]
            if mt % 2 == 1:
                del xt_tiles[mt // 2]
            c, mi = chunk_of(mt)
            if c not in s_cols:
                s_cols[c] = stat.tile(
                    [P, MAX_CHUNK_MT], F32, tag="s_col", name="s_col"
                )

            amax = stat.tile([P, 1], F32, tag="amax", name="amax")
            nc.vector.tensor_reduce(
                amax[:], xt[:, mt % 2, :], axis=mybir.AxisListType.X,
                op=mybir.AluOpType.max, apply_absolute_value=True,
            )
            nc.vector.tensor_scalar_max(amax[:], amax[:], 1e-5)
            q = stat.tile([P, 1], F32, tag="q", name="q")
            nc.vector.reciprocal(q[:], amax[:])
            s_bf = stat.tile([P, 1], BF16, tag="s_bf", name="s_bf")
            nc.vector.tensor_scalar_mul(s_bf[:], q[:], 127.0)
            s_f32 = s_cols[c][:, mi : mi + 1]
            nc.vector.tensor_copy(s_f32, s_bf[:])
            r0 = stat.tile([P, 1], F32, tag="r0", name="r0")
            nc.vector.reciprocal(r0[:], s_f32)
            nc.vector.tensor_tensor(
                r_all[:, mt : mt + 1], r0[:], iws_bc[:], op=mybir.AluOpType.mult
            )

        def store_s_chunk(c):
            cmt = CHUNK_MTS[c]
            c0 = CHUNK_STARTS[c]
            nc.gpsimd.dma_start(
                s_dram[c0 : c0 + cmt, :].rearrange("j p -> p j"),
                s_cols.pop(c)[:, :cmt],
            )

        def srow_sbc(c):
            cmt = CHUNK_MTS[c]
            cw = cmt * P
            c0 = CHUNK_STARTS[c]
            srow = srow_pool.tile([1, MAX_CHUNK_MT * P], F32, tag="srow", name="srow")
            nc.gpsimd.dma_start(
                srow[0:1, :cw],
                s_dram[c0 : c0 + cmt, :].rearrange("j p -> (j p)").unsqueeze(0),
            )
            sbc = sbc_pool.tile([P, MAX_CHUNK_MT * P], F32, tag="sbc", name="sbc")
            nc.gpsimd.partition_broadcast(sbc[:, :cw], srow[0:1, :cw])
            return sbc

        def quant_chunk(c, xT, sbc):
            cw = CHUNK_MTS[c] * P
            tiles = []
            for kc in range(KC):
                src = xT[kc // KCH][:, kc % KCH, :cw]
                q8 = xq8_pool.tile(
                    [P, MAX_CHUNK_MT * P], I8, tag="xq8", name="xq8"
                )
                nc.vector.tensor_tensor(
                    q8[:, :cw], src, sbc[:, :cw], op=mybir.AluOpType.mult
                )
                qb = xqbf_pool.tile(
                    [P, MAX_CHUNK_MT * P], BF16, tag=f"xqbf{kc}", name=f"xqbf{kc}"
                )
                nc.vector.tensor_copy(qb[:, :cw], q8[:, :cw])
                tiles.append(qb)
            return tiles

        def matmul_mtile(c, mi, xqbf):
            mt = CHUNK_STARTS[c] + mi
            y_sb = yout.tile([P, N], BF16, tag="y_sb", name="y_sb")
            for nt in range(NTN):
                ps = psum.tile([P, N_TILE], F32, tag="ps", name="ps")
                for kc in range(KC):
                    nc.tensor.matmul(
                        ps[:],
                        xqbf[kc][:, mi * P : (mi + 1) * P],
                        wt[:, kc, nt * N_TILE : (nt + 1) * N_TILE],
                        start=(kc == 0),
                        stop=(kc == KC - 1),
                    )
                nc.scalar.activation(
                    y_sb[:, nt * N_TILE : (nt + 1) * N_TILE],
                    ps[:],
                    mybir.ActivationFunctionType.Copy,
                    scale=r_all[:, mt : mt + 1],
                )
            nc.gpsimd.dma_start(y_tiled[mt], y_sb[:])

        def amax_chunk(c):
            for mt in range(CHUNK_STARTS[c], CHUNK_STARTS[c] + CHUNK_MTS[c]):
                amax_tile(mt)
            store_s_chunk(c)

        # software pipeline: transposes/quant of chunk c+1 and amax of
        # chunk c+3 run during chunk c's matmuls.
        amax_chunk(0)
        amax_chunk(1)
        sbcs = {0: srow_sbc(0), 1: srow_sbc(1)}
        xq_map = {0: quant_chunk(0, t0, sbcs.pop(0))}
        amax_chunk(2)
        sbcs[2] = srow_sbc(2)
        for c in range(CHUNKS):
            if c + 1 < CHUNKS:
                tc1 = transpose_chunk(c + 1)
                xq_map[c + 1] = quant_chunk(c + 1, tc1, sbcs.pop(c + 1))
            if c + 3 < CHUNKS:
                amax_chunk(c + 3)
                sbcs[c + 3] = srow_sbc(c + 3)
            for mi in range(CHUNK_MTS[c]):
                matmul_mtile(c, mi, xq_map[c])
            del xq_map[c]

    nc.compile()
    return nc


def unpack_wT_f8(packed_weight: np.ndarray) -> np.ndarray:
    planes = [((packed_weight >> (2 * i)) & 3) for i in range(4)]
    w = np.concatenate(planes, axis=0).astype(np.float32) - 1.0  # [N, K] ternary
    return np.ascontiguousarray(w.T).astype(ml_dtypes.float8_e4m3)  # [K, N] fp8


_CACHE = {}


def run(x: np.ndarray, packed_weight: np.ndarray, weight_scale: np.ndarray,
        trace: bool = False, tmpdir=None):
    """x: [B, S, K] bf16 -> y [B, S, N] bf16 (full, unsharded)."""
    if "nc" not in _CACHE:
        _CACHE["nc"] = build_kernel()
    nc = _CACHE["nc"]

    B, S, D = x.shape
    M = B * S
    assert M == M_LOC * N_CORES and D == K
    wT = unpack_wT_f8(packed_weight)
    iws = np.asarray(
        1.0 / np.float32(np.asarray(weight_scale).reshape(-1)[0]), dtype=np.float32
    ).reshape(1, 1)
    shards = np.ascontiguousarray(np.asarray(x).reshape(N_CORES, M_LOC, K))
    in_maps = [
        {"x": shards[i], "wTf8": wT, "inv_ws": iws} for i in range(N_CORES)
    ]
    res = bass_utils.run_bass_kernel_spmd(
        nc, in_maps, core_ids=list(range(N_CORES)), trace=trace, tmpdir=tmpdir
    )
    y = np.stack([res.results[i]["y"] for i in range(N_CORES)], axis=0)
    return y.reshape(B, S, N), res


def kernel(x, packed_weight, weight_scale):
    """Harness entrypoint: FULL inputs -> FULL output.

    x: [4, 8192, 2048] bf16; packed_weight: [512, 2048] uint8;
    weight_scale: [1] bf16.  Returns [4, 8192, 2048] bf16.
    Sharding: data-parallel over tokens across the 8 NeuronCores;
    the (host-unpacked) ternary weight is replicated.
    """
    x = np.asarray(x)
    packed_weight = np.asarray(packed_weight)
    weight_scale = np.asarray(weight_scale)
    y, _ = run(x, packed_weight, weight_scale)
    return y
